# revision 39
# baseline (speedup 1.0000x reference)
"""Sparse graph-attention kernel for 8 TRN2 NeuronCores (Bass/Tile).

Problem (hardcoded): N=20000 nodes, E=640000 edges (src-sorted), Fin=256,
Fqk=256.  out[e] = exp(aw[e]) / segsum_src(exp(aw)),
aw[e] = (x[src[e]] @ Wq.T * Fqk**-0.5) . (x[dest[e]] @ Wk.T).

Key identity: aw[e] = p[src[e]] . x[dest[e]] with p = x @ G,
G = Fqk**-0.5 * Wq.T @ Wk (weight-only fold, computed host-side).

Transport: the full x table is held in SBUF as an int32-paired feature-major
table ktT[p, n] = (bf16 x[n, 2p], bf16 x[n, 2p+1]), so one gpsimd ap_gather
element per edge moves the whole 512B k-row (features across partitions).
Edges are packed per src node into capacity-class columns (capacity =
ceil(deg/4)*4, schedule = pointwise max of per-core sorted class lists, so
one compiled graph serves all cores); per node, two PE matmuls (even/odd
feature planes, stride-2 lhsT over the gathered pairs) against the node's
projected p-column produce the per-edge dots directly in PSUM [deg, col] --
no per-slot q expansion and no elementwise multiply pass.  Softmax per
column: mask-add, exp (Act), partition-sum via ones-matmul, reciprocal,
K=1-matmul broadcast, and one elementwise divide.

Sharding: src-node ranges (2500 nodes/core); each core gathers its dest
rows from the replicated SBUF table.
"""

import numpy as np
import ml_dtypes

N = 20000
E = 640000
FIN = 256
FQK = 256
NCORES = 8
NLOC = N // NCORES          # 2500 nodes per core
CL = 1                      # capacity class granularity (exact degrees)
NCH = 4                     # gather chunks (each >= N idxs for full rate)
QROWS = 2560                # pT/xlT column capacity (>= NLOC)
P = 128
REG = 256                   # softmax region width (PSUM cols)

bf16 = ml_dtypes.bfloat16
_compiled = None
_sched = None               # (slotcls, chunk col ranges, chunk slot counts, offsets)


def _wrap_idx(vals):
    """int16 vals [n] (n % 16 == 0) -> ap_gather idx layout [128, n/16]:
    idx j -> partition j%16 (replicated across the 8 groups), col j//16."""
    n = vals.shape[0]
    a = vals.reshape(n // 16, 16).T                      # [16, n/16]
    return np.ascontiguousarray(np.tile(a, (8, 1)).astype(np.int16))


def _schedule(all_counts):
    """Static schedule from per-core degree lists (identical for all cores).

    Columns (one per node rank, ascending capacity) are packed into GROUPS
    of consecutive columns with total capacity <= 128 (one matmul pair per
    group: out [S, g] psum block, off-diagonal entries masked); groups pack
    into NCH gather chunks (each >= N idxs for full ap_gather rate).

    Returns (slotcls [NLOC], chunks, sloto [NLOC], rowoff [NLOC]) with
    chunks = [(c0, c1, nsl, groups)], groups = [(i0, i1, goff, S)];
    sloto[i] = column i's slot offset in its chunk, rowoff[i] = column i's
    first row (partition) inside its group's psum block."""
    slotcls = np.zeros(NLOC, np.int64)
    for c in range(NCORES):
        d = all_counts[c * NLOC:(c + 1) * NLOC]
        cls = np.sort(-(-(d) // CL) * CL)[::-1]          # descending classes
        assert cls.max() <= P, "node degree exceeds one PSUM column"
        slotcls = np.maximum(slotcls, cls)
    slotcls = slotcls[::-1].copy()                       # ascending
    # groups of consecutive columns, capacity sum <= 128
    groups = []
    i = 0
    rowoff = np.zeros(NLOC, np.int64)
    while i < NLOC:
        j, s = i, 0
        while j < NLOC and s + slotcls[j] <= P:
            rowoff[j] = s
            s += int(slotcls[j])
            j += 1
        groups.append((i, j, s))                         # cols [i, j), S slots
        i = j
    # chunk boundaries at group granularity, ~equal slots
    tot = sum(s for _, _, s in groups)
    chunks = []
    sloto = np.zeros(NLOC, np.int64)
    gi = 0
    acc_target = 0
    for k in range(NCH):
        acc_target += tot / NCH
        glist = []
        off = 0
        c0 = groups[gi][0]
        while gi < len(groups):
            i0, i1, s = groups[gi]
            for i in range(i0, i1):
                sloto[i] = off + rowoff[i]
            glist.append((i0, i1, off, s))
            off += s
            gi += 1
            done = sum(ss for _, _, ss in groups[:gi])
            if k < NCH - 1 and done >= acc_target:
                break
        c1 = glist[-1][1]
        nsl = -(-off // 16) * 16                         # pad to 16
        assert nsl >= N, "chunk below table-size floor; retune NCH"
        chunks.append((c0, c1, nsl, glist))
    return slotcls, chunks, sloto, rowoff


def _host_prep(x, ei, W):
    global _sched
    src = np.asarray(ei[0], np.int64)
    dest = np.asarray(ei[1], np.int64)
    x = np.asarray(x, np.float32)
    W = np.asarray(W, np.float32)

    # weight-only fold: aw[e] = (x[src] @ G) . x[dest]; even/odd G columns
    G = (FQK ** -0.5) * (W[:FQK].T @ W[FQK:])            # [256, 256]
    Ge = np.ascontiguousarray(G[:, 0::2].astype(bf16))   # [256, 128]
    Go = np.ascontiguousarray(G[:, 1::2].astype(bf16))

    # feature-major int32-paired gather table (same for all cores)
    xb = x.astype(bf16)                                  # [N, 256]
    ktT = np.ascontiguousarray(
        xb.reshape(N, P, 2).transpose(1, 0, 2)).view(np.int32).reshape(P, N)

    counts = np.bincount(src, minlength=N)
    starts = np.concatenate([[0], np.cumsum(counts)])    # [N+1]

    slotcls, chunks, sloto, rowoff = _schedule(counts)
    _sched = (slotcls, chunks, sloto, rowoff)
    totslots = sum(nsl for _, _, nsl, _ in chunks)

    in_maps = []
    unshard = []
    for c in range(NCORES):
        n0 = c * NLOC
        d = counts[n0:n0 + NLOC]
        cls = -(-d // CL) * CL
        order = np.argsort(-cls, kind="stable")[::-1]    # rank i -> local node
        assert (cls[order] <= slotcls).all(), "schedule infeasible"

        wraps = []
        for (c0, c1, nsl, _) in chunks:
            seg = np.zeros(nsl, np.int16)
            for i in range(c0, c1):
                n = n0 + order[i]
                dg = int(counts[n])
                o = int(sloto[i])
                seg[o:o + dg] = dest[starts[n]:starts[n] + dg].astype(np.int16)
            wraps.append(_wrap_idx(seg))
        kgi_w = np.ascontiguousarray(np.concatenate(wraps, axis=1))

        xl = np.zeros((QROWS, FIN), np.float32)
        xl[:NLOC] = x[n0 + order]
        xlT = np.ascontiguousarray(xl.T.astype(bf16))    # [256, QROWS]

        mB = np.full((P, QROWS), -30.0, np.float32)
        for i in range(NLOC):
            r = int(rowoff[i])
            mB[r:r + d[order[i]], i] = 0.0

        in_maps.append(dict(ktT=ktT, kgi=kgi_w, xlT=xlT, Ge=Ge, Go=Go, mB=mB))
        unshard.append((order, starts[n0:n0 + NLOC + 1].copy(), d, rowoff))
    return in_maps, unshard


def _build():
    import concourse.bacc as bacc
    import concourse.mybir as mybir
    import concourse.tile as tile
    from concourse import library_config
    from concourse.tile_rust import add_dep_helper

    fp32 = mybir.dt.float32
    b16 = mybir.dt.bfloat16
    i32 = mybir.dt.int32
    i16 = mybir.dt.int16
    Alu = mybir.AluOpType

    slotcls, chunks, sloto, rowoff = _sched
    totslots = sum(nsl for _, _, nsl, _ in chunks)
    maxch = max(nsl for _, _, nsl, _ in chunks)

    nc = bacc.Bacc("TRN2", target_bir_lowering=False, debug=False)
    ktT_d = nc.dram_tensor("ktT", [P, N], i32, kind="ExternalInput")
    kgi_d = nc.dram_tensor("kgi", [P, totslots // 16], i16, kind="ExternalInput")
    xlT_d = nc.dram_tensor("xlT", [FIN, QROWS], b16, kind="ExternalInput")
    Ge_d = nc.dram_tensor("Ge", [FIN, P], b16, kind="ExternalInput")
    Go_d = nc.dram_tensor("Go", [FIN, P], b16, kind="ExternalInput")
    mB_d = nc.dram_tensor("mB", [P, QROWS], fp32, kind="ExternalInput")
    out_d = nc.dram_tensor("out", [P, QROWS], fp32, kind="ExternalOutput")

    with tile.TileContext(nc) as tc:
        with tc.tile_pool(name="persist", bufs=1) as sb, \
             tc.tile_pool(name="ktp", bufs=1) as ktp, \
             tc.tile_pool(name="kgip", bufs=2) as kgip, \
             tc.tile_pool(name="reg", bufs=2) as rp, \
             tc.tile_pool(name="exp", bufs=4) as exp_p, \
             tc.tile_pool(name="outp", bufs=3) as outp, \
             tc.tile_pool(name="recp", bufs=2) as recp, \
             tc.tile_pool(name="aws", bufs=4, space="PSUM") as awps, \
             tc.tile_pool(name="esps", bufs=2, space="PSUM") as esps, \
             tc.tile_pool(name="rbps", bufs=2, space="PSUM") as rbps:
            lib = nc.gpsimd.load_library(library_config.ap_gather)

            # --- persistent inputs; table load split SP/Act/Pool (the only
            # DMA-capable engines); non-critical loads follow the table ---
            ktT = sb.tile([P, N, 1], i32)
            b0, b1 = 6550, 12750
            nc.sync.dma_start(ktT[:, :b0, :],
                              ktT_d[:, :b0].rearrange("p (n d) -> p n d", d=1))
            nc.scalar.dma_start(ktT[:, b0:b1, :],
                                ktT_d[:, b0:b1].rearrange("p (n d) -> p n d", d=1))
            nc.gpsimd.dma_start(ktT[:, b1:, :],
                                ktT_d[:, b1:].rearrange("p (n d) -> p n d", d=1))
            xlT = sb.tile([P, 2, QROWS], b16)
            nc.scalar.dma_start(xlT[:], xlT_d[:, :].rearrange("(c p) f -> p c f", p=P))
            Ge = sb.tile([P, 2, P], b16)
            Go = sb.tile([P, 2, P], b16)
            nc.scalar.dma_start(Ge[:], Ge_d[:, :].rearrange("(c p) f -> p c f", p=P))
            nc.scalar.dma_start(Go[:], Go_d[:, :].rearrange("(c p) f -> p c f", p=P))
            mB = sb.tile([P, QROWS], fp32)
            nc.scalar.dma_start(mB[:], mB_d[:])
            onesf = sb.tile([P, 1], fp32)
            nc.vector.memset(onesf[:], 1.0)
            ones1 = sb.tile([1, P], fp32)
            nc.vector.memset(ones1[:], 1.0)
            maxch16 = maxch // 16

            # --- p projection in even/odd planes: pT_u[p, i] = q[i, 2p+u] ---
            pTe = sb.tile([P, QROWS], b16)
            pTo = sb.tile([P, QROWS], b16)
            for u, (Gt, pT) in enumerate(((Ge, pTe), (Go, pTo))):
                for o in range(0, QROWS, REG):
                    acc = awps.tile([P, REG], fp32, tag="aw")
                    nc.tensor.matmul(acc[:], lhsT=Gt[:, 0, :],
                                     rhs=xlT[:, 0, o:o + REG], start=True, stop=False)
                    nc.tensor.matmul(acc[:], lhsT=Gt[:, 1, :],
                                     rhs=xlT[:, 1, o:o + REG], start=False, stop=True)
                    nc.scalar.copy(pT[:, o:o + REG], acc[:])

            # --- main stream: per chunk, one ap_gather then grouped dots ---
            soff16 = 0
            for k, (c0, c1, nsl, glist) in enumerate(chunks):
                kgi_t = kgip.tile([P, maxch16], i16, tag="kgi")
                nc.sync.dma_start(kgi_t[:, :nsl // 16],
                                  kgi_d[:, soff16 // 16:(soff16 + nsl) // 16])
                kt = ktp.tile([P, maxch, 1], i32, tag="kt")
                g = nc.gpsimd.ap_gather(
                    kt[:, :nsl, :], ktT[:], kgi_t[:, :nsl // 16],
                    P, N, 1, nsl)
                add_dep_helper(lib.ins, g.ins, sync=True, reason="lib first")
                soff16 += nsl
                ktv = kt[:].bitcast(b16)                 # [P, maxch, 2]

                # regions = runs of whole groups, <= REG columns each;
                # per-chunk phase ordering so no in-order engine queues a
                # blocked op ahead of ready ones (DVE div waits on PE rb)
                rw = REG if k < NCH - 1 else 1 + (c1 - c0) // 3
                regions = []
                cur = []
                for grp in glist:
                    if cur and grp[1] - cur[0][0] > rw:
                        regions.append(cur)
                        cur = []
                    cur.append(grp)
                if cur:
                    regions.append(cur)
                rd = []
                for rgs in regions:
                    r0, r1 = rgs[0][0], rgs[-1][1]
                    aw = awps.tile([P, REG], fp32, tag="aw")
                    nc.vector.memset(aw[:], -30.0)
                    for (i0, i1, goff, S) in rgs:
                        gw = i1 - i0
                        j = i0 - r0
                        nc.tensor.matmul(aw[0:S, j:j + gw],
                                         lhsT=ktv[:, goff:goff + S, 0],
                                         rhs=pTe[:, i0:i1],
                                         start=True, stop=False)
                        nc.tensor.matmul(aw[0:S, j:j + gw],
                                         lhsT=ktv[:, goff:goff + S, 1],
                                         rhs=pTo[:, i0:i1],
                                         start=False, stop=True)
                    rd.append(dict(r0=r0, w=r1 - r0, aw=aw))
                for r in rd:
                    r["awm"] = rp.tile([P, REG], fp32, tag="awm", name="awm_t")
                    nc.vector.tensor_tensor(out=r["awm"][:, :r["w"]],
                                            in0=r["aw"][:, :r["w"]],
                                            in1=mB[:, r["r0"]:r["r0"] + r["w"]],
                                            op=Alu.add)
                for r in rd:
                    r["ex"] = exp_p.tile([P, REG], fp32, tag="ex", name="ex_t")
                    nc.scalar.activation(r["ex"][:, :r["w"]], r["awm"][:, :r["w"]],
                                         mybir.ActivationFunctionType.Exp)
                for r in rd:
                    r["es"] = esps.tile([1, REG], fp32, tag="es", name="es_t")
                    nc.tensor.matmul(r["es"][0:1, :r["w"]], lhsT=onesf[:, 0:1],
                                     rhs=r["ex"][:, :r["w"]], start=True, stop=True)
                for r in rd:
                    r["rec"] = recp.tile([1, REG], fp32, tag="rec", name="rec_t")
                    nc.vector.reciprocal_approx_fast(r["rec"][0:1, :r["w"]],
                                                     r["es"][0:1, :r["w"]])
                for r in rd:
                    r["rb"] = rbps.tile([P, REG], fp32, tag="rb", name="rb_t")
                    nc.tensor.matmul(r["rb"][:, :r["w"]], lhsT=ones1[:],
                                     rhs=r["rec"][0:1, :r["w"]], start=True, stop=True)
                for r in rd:
                    r["o"] = outp.tile([P, REG], fp32, tag="o", name="o_t")
                    nc.vector.tensor_tensor(out=r["o"][:, :r["w"]],
                                            in0=r["ex"][:, :r["w"]],
                                            in1=r["rb"][:, :r["w"]], op=Alu.mult)
                for q, r in enumerate(rd):
                    eng = nc.sync if q % 2 == 0 else nc.scalar
                    eng.dma_start(out_d[:, r["r0"]:r["r0"] + r["w"]],
                                  r["o"][:, :r["w"]])
    nc.compile()
    return nc


def kernel(x, ei, W):
    global _compiled
    in_maps, unshard = _host_prep(x, ei, W)
    if _compiled is None:
        _compiled = _build()
    nc = _compiled
    from concourse.bass_utils import run_bass_kernel_spmd
    res = run_bass_kernel_spmd(nc, in_maps, core_ids=list(range(NCORES)))
    out = np.empty(E, np.float32)
    for c in range(NCORES):
        order, starts_l, d, rowoff = unshard[c]
        o = res.results[c]["out"]
        for i in range(NLOC):
            n = order[i]
            dg = int(d[n])
            r = int(rowoff[i])
            out[starts_l[n]:starts_l[n] + dg] = o[r:r + dg, i]
    return out


# revision 40
# speedup vs baseline: 1.0024x; 1.0024x over previous
"""Sparse graph-attention kernel for 8 TRN2 NeuronCores (Bass/Tile).

Problem (hardcoded): N=20000 nodes, E=640000 edges (src-sorted), Fin=256,
Fqk=256.  out[e] = exp(aw[e]) / segsum_src(exp(aw)),
aw[e] = (x[src[e]] @ Wq.T * Fqk**-0.5) . (x[dest[e]] @ Wk.T).

Key identity: aw[e] = p[src[e]] . x[dest[e]] with p = x @ G,
G = Fqk**-0.5 * Wq.T @ Wk (weight-only fold, computed host-side).

Transport: the full x table is held in SBUF as an int32-paired feature-major
table ktT[p, n] = (bf16 x[n, 2p], bf16 x[n, 2p+1]), so one gpsimd ap_gather
element per edge moves the whole 512B k-row (features across partitions).
Edges are packed per src node into capacity-class columns (capacity =
ceil(deg/4)*4, schedule = pointwise max of per-core sorted class lists, so
one compiled graph serves all cores); per node, two PE matmuls (even/odd
feature planes, stride-2 lhsT over the gathered pairs) against the node's
projected p-column produce the per-edge dots directly in PSUM [deg, col] --
no per-slot q expansion and no elementwise multiply pass.  Softmax per
column: mask-add, exp (Act), partition-sum via ones-matmul, reciprocal,
K=1-matmul broadcast, and one elementwise divide.

Sharding: src-node ranges (2500 nodes/core); each core gathers its dest
rows from the replicated SBUF table.
"""

import numpy as np
import ml_dtypes

N = 20000
E = 640000
FIN = 256
FQK = 256
NCORES = 8
NLOC = N // NCORES          # 2500 nodes per core
CL = 1                      # capacity class granularity (exact degrees)
NCH = 4                     # gather chunks (each >= N idxs for full rate)
QROWS = 2560                # pT/xlT column capacity (>= NLOC)
P = 128
REG = 256                   # softmax region width (PSUM cols)

bf16 = ml_dtypes.bfloat16
_compiled = None
_sched = None               # (slotcls, chunk col ranges, chunk slot counts, offsets)


def _wrap_idx(vals):
    """int16 vals [n] (n % 16 == 0) -> ap_gather idx layout [128, n/16]:
    idx j -> partition j%16 (replicated across the 8 groups), col j//16."""
    n = vals.shape[0]
    a = vals.reshape(n // 16, 16).T                      # [16, n/16]
    return np.ascontiguousarray(np.tile(a, (8, 1)).astype(np.int16))


def _schedule(all_counts):
    """Static schedule from per-core degree lists (identical for all cores).

    Columns (one per node rank, ascending capacity) are packed into GROUPS
    of consecutive columns with total capacity <= 128 (one matmul pair per
    group: out [S, g] psum block, off-diagonal entries masked); groups pack
    into NCH gather chunks (each >= N idxs for full ap_gather rate).

    Returns (slotcls [NLOC], chunks, sloto [NLOC], rowoff [NLOC]) with
    chunks = [(c0, c1, nsl, groups)], groups = [(i0, i1, goff, S)];
    sloto[i] = column i's slot offset in its chunk, rowoff[i] = column i's
    first row (partition) inside its group's psum block."""
    slotcls = np.zeros(NLOC, np.int64)
    for c in range(NCORES):
        d = all_counts[c * NLOC:(c + 1) * NLOC]
        cls = np.sort(-(-(d) // CL) * CL)[::-1]          # descending classes
        assert cls.max() <= P, "node degree exceeds one PSUM column"
        slotcls = np.maximum(slotcls, cls)
    slotcls = slotcls[::-1].copy()                       # ascending
    # groups of consecutive columns, capacity sum <= 128
    groups = []
    i = 0
    rowoff = np.zeros(NLOC, np.int64)
    while i < NLOC:
        j, s = i, 0
        while j < NLOC and s + slotcls[j] <= P:
            rowoff[j] = s
            s += int(slotcls[j])
            j += 1
        groups.append((i, j, s))                         # cols [i, j), S slots
        i = j
    # chunk boundaries at group granularity, ~equal slots
    tot = sum(s for _, _, s in groups)
    chunks = []
    sloto = np.zeros(NLOC, np.int64)
    gi = 0
    acc_target = 0
    for k in range(NCH):
        acc_target += tot / NCH
        glist = []
        off = 0
        c0 = groups[gi][0]
        while gi < len(groups):
            i0, i1, s = groups[gi]
            for i in range(i0, i1):
                sloto[i] = off + rowoff[i]
            glist.append((i0, i1, off, s))
            off += s
            gi += 1
            done = sum(ss for _, _, ss in groups[:gi])
            if k < NCH - 1 and done >= acc_target:
                break
        c1 = glist[-1][1]
        nsl = -(-off // 16) * 16                         # pad to 16
        assert nsl >= N, "chunk below table-size floor; retune NCH"
        chunks.append((c0, c1, nsl, glist))
    return slotcls, chunks, sloto, rowoff


def _host_prep(x, ei, W):
    global _sched
    src = np.asarray(ei[0], np.int64)
    dest = np.asarray(ei[1], np.int64)
    x = np.asarray(x, np.float32)
    W = np.asarray(W, np.float32)

    # weight-only fold: aw[e] = (x[src] @ G) . x[dest]; even/odd G columns
    G = (FQK ** -0.5) * (W[:FQK].T @ W[FQK:])            # [256, 256]
    Ge = np.ascontiguousarray(G[:, 0::2].astype(bf16))   # [256, 128]
    Go = np.ascontiguousarray(G[:, 1::2].astype(bf16))

    # feature-major int32-paired gather table (same for all cores)
    xb = x.astype(bf16)                                  # [N, 256]
    ktT = np.ascontiguousarray(
        xb.reshape(N, P, 2).transpose(1, 0, 2)).view(np.int32).reshape(P, N)

    counts = np.bincount(src, minlength=N)
    starts = np.concatenate([[0], np.cumsum(counts)])    # [N+1]

    slotcls, chunks, sloto, rowoff = _schedule(counts)
    _sched = (slotcls, chunks, sloto, rowoff)
    totslots = sum(nsl for _, _, nsl, _ in chunks)

    in_maps = []
    unshard = []
    for c in range(NCORES):
        n0 = c * NLOC
        d = counts[n0:n0 + NLOC]
        cls = -(-d // CL) * CL
        order = np.argsort(-cls, kind="stable")[::-1]    # rank i -> local node
        assert (cls[order] <= slotcls).all(), "schedule infeasible"

        wraps = []
        for (c0, c1, nsl, _) in chunks:
            seg = np.zeros(nsl, np.int16)
            for i in range(c0, c1):
                n = n0 + order[i]
                dg = int(counts[n])
                o = int(sloto[i])
                seg[o:o + dg] = dest[starts[n]:starts[n] + dg].astype(np.int16)
            wraps.append(_wrap_idx(seg))
        kgi_w = np.ascontiguousarray(np.concatenate(wraps, axis=1))

        xl = np.zeros((QROWS, FIN), np.float32)
        xl[:NLOC] = x[n0 + order]
        xlT = np.ascontiguousarray(xl.T.astype(bf16))    # [256, QROWS]

        mB = np.full((P, QROWS), -30.0, np.float32)
        for i in range(NLOC):
            r = int(rowoff[i])
            mB[r:r + d[order[i]], i] = 0.0

        in_maps.append(dict(ktT=ktT, kgi=kgi_w, xlT=xlT, Ge=Ge, Go=Go, mB=mB))
        unshard.append((order, starts[n0:n0 + NLOC + 1].copy(), d, rowoff))
    return in_maps, unshard


def _build():
    import concourse.bacc as bacc
    import concourse.mybir as mybir
    import concourse.tile as tile
    from concourse import library_config
    from concourse.tile_rust import add_dep_helper

    fp32 = mybir.dt.float32
    b16 = mybir.dt.bfloat16
    i32 = mybir.dt.int32
    i16 = mybir.dt.int16
    Alu = mybir.AluOpType

    slotcls, chunks, sloto, rowoff = _sched
    totslots = sum(nsl for _, _, nsl, _ in chunks)
    maxch = max(nsl for _, _, nsl, _ in chunks)

    nc = bacc.Bacc("TRN2", target_bir_lowering=False, debug=False)
    ktT_d = nc.dram_tensor("ktT", [P, N], i32, kind="ExternalInput")
    kgi_d = nc.dram_tensor("kgi", [P, totslots // 16], i16, kind="ExternalInput")
    xlT_d = nc.dram_tensor("xlT", [FIN, QROWS], b16, kind="ExternalInput")
    Ge_d = nc.dram_tensor("Ge", [FIN, P], b16, kind="ExternalInput")
    Go_d = nc.dram_tensor("Go", [FIN, P], b16, kind="ExternalInput")
    mB_d = nc.dram_tensor("mB", [P, QROWS], fp32, kind="ExternalInput")
    out_d = nc.dram_tensor("out", [P, QROWS], fp32, kind="ExternalOutput")

    with tile.TileContext(nc) as tc:
        with tc.tile_pool(name="persist", bufs=1) as sb, \
             tc.tile_pool(name="ktp", bufs=1) as ktp, \
             tc.tile_pool(name="kgip", bufs=2) as kgip, \
             tc.tile_pool(name="reg", bufs=2) as rp, \
             tc.tile_pool(name="exp", bufs=4) as exp_p, \
             tc.tile_pool(name="outp", bufs=3) as outp, \
             tc.tile_pool(name="recp", bufs=2) as recp, \
             tc.tile_pool(name="aws", bufs=4, space="PSUM") as awps, \
             tc.tile_pool(name="esps", bufs=2, space="PSUM") as esps, \
             tc.tile_pool(name="rbps", bufs=2, space="PSUM") as rbps:
            lib = nc.gpsimd.load_library(library_config.ap_gather)

            # --- persistent inputs; table load split SP/Act/Pool (the only
            # DMA-capable engines); non-critical loads follow the table ---
            ktT = sb.tile([P, N, 1], i32)
            b0, b1 = 6550, 12750
            nc.sync.dma_start(ktT[:, :b0, :],
                              ktT_d[:, :b0].rearrange("p (n d) -> p n d", d=1))
            nc.scalar.dma_start(ktT[:, b0:b1, :],
                                ktT_d[:, b0:b1].rearrange("p (n d) -> p n d", d=1))
            nc.gpsimd.dma_start(ktT[:, b1:, :],
                                ktT_d[:, b1:].rearrange("p (n d) -> p n d", d=1))
            xlT = sb.tile([P, 2, QROWS], b16)
            nc.scalar.dma_start(xlT[:], xlT_d[:, :].rearrange("(c p) f -> p c f", p=P))
            Ge = sb.tile([P, 2, P], b16)
            Go = sb.tile([P, 2, P], b16)
            nc.scalar.dma_start(Ge[:], Ge_d[:, :].rearrange("(c p) f -> p c f", p=P))
            nc.scalar.dma_start(Go[:], Go_d[:, :].rearrange("(c p) f -> p c f", p=P))
            mB = sb.tile([P, QROWS], fp32)
            nc.scalar.dma_start(mB[:], mB_d[:])
            onesf = sb.tile([P, 1], fp32)
            nc.vector.memset(onesf[:], 1.0)
            ones1 = sb.tile([1, P], fp32)
            nc.vector.memset(ones1[:], 1.0)
            maxch16 = maxch // 16

            # --- p projection in even/odd planes: pT_u[p, i] = q[i, 2p+u] ---
            pTe = sb.tile([P, QROWS], b16)
            pTo = sb.tile([P, QROWS], b16)
            for u, (Gt, pT) in enumerate(((Ge, pTe), (Go, pTo))):
                for o in range(0, QROWS, REG):
                    acc = awps.tile([P, REG], fp32, tag="aw")
                    nc.tensor.matmul(acc[:], lhsT=Gt[:, 0, :],
                                     rhs=xlT[:, 0, o:o + REG], start=True, stop=False)
                    nc.tensor.matmul(acc[:], lhsT=Gt[:, 1, :],
                                     rhs=xlT[:, 1, o:o + REG], start=False, stop=True)
                    nc.scalar.copy(pT[:, o:o + REG], acc[:])

            # --- main stream: per chunk, one ap_gather then grouped dots ---
            soff16 = 0
            for k, (c0, c1, nsl, glist) in enumerate(chunks):
                kgi_t = kgip.tile([P, maxch16], i16, tag="kgi")
                nc.sync.dma_start(kgi_t[:, :nsl // 16],
                                  kgi_d[:, soff16 // 16:(soff16 + nsl) // 16])
                kt = ktp.tile([P, maxch, 1], i32, tag="kt")
                g = nc.gpsimd.ap_gather(
                    kt[:, :nsl, :], ktT[:], kgi_t[:, :nsl // 16],
                    P, N, 1, nsl)
                add_dep_helper(lib.ins, g.ins, sync=True, reason="lib first")
                soff16 += nsl
                ktv = kt[:].bitcast(b16)                 # [P, maxch, 2]

                # regions = runs of whole groups, <= REG columns each;
                # per-chunk phase ordering so no in-order engine queues a
                # blocked op ahead of ready ones (DVE div waits on PE rb)
                if k < NCH - 1:
                    rcaps = [REG] * 16
                else:
                    w0 = c1 - c0
                    rcaps = [(2 * w0) // 5, (2 * w0) // 5, w0]
                regions = []
                cur = []
                ci = 0
                for grp in glist:
                    cap = rcaps[min(ci, len(rcaps) - 1)]
                    if cur and grp[1] - cur[0][0] > cap:
                        regions.append(cur)
                        cur = []
                        ci += 1
                    cur.append(grp)
                if cur:
                    regions.append(cur)
                rd = []
                for rgs in regions:
                    r0, r1 = rgs[0][0], rgs[-1][1]
                    aw = awps.tile([P, REG], fp32, tag="aw")
                    nc.vector.memset(aw[:], -30.0)
                    for (i0, i1, goff, S) in rgs:
                        gw = i1 - i0
                        j = i0 - r0
                        nc.tensor.matmul(aw[0:S, j:j + gw],
                                         lhsT=ktv[:, goff:goff + S, 0],
                                         rhs=pTe[:, i0:i1],
                                         start=True, stop=False)
                        nc.tensor.matmul(aw[0:S, j:j + gw],
                                         lhsT=ktv[:, goff:goff + S, 1],
                                         rhs=pTo[:, i0:i1],
                                         start=False, stop=True)
                    rd.append(dict(r0=r0, w=r1 - r0, aw=aw))
                for r in rd:
                    r["awm"] = rp.tile([P, REG], fp32, tag="awm", name="awm_t")
                    nc.vector.tensor_tensor(out=r["awm"][:, :r["w"]],
                                            in0=r["aw"][:, :r["w"]],
                                            in1=mB[:, r["r0"]:r["r0"] + r["w"]],
                                            op=Alu.add)
                for r in rd:
                    r["ex"] = exp_p.tile([P, REG], fp32, tag="ex", name="ex_t")
                    nc.scalar.activation(r["ex"][:, :r["w"]], r["awm"][:, :r["w"]],
                                         mybir.ActivationFunctionType.Exp)
                for r in rd:
                    r["es"] = esps.tile([1, REG], fp32, tag="es", name="es_t")
                    nc.tensor.matmul(r["es"][0:1, :r["w"]], lhsT=onesf[:, 0:1],
                                     rhs=r["ex"][:, :r["w"]], start=True, stop=True)
                for r in rd:
                    r["rec"] = recp.tile([1, REG], fp32, tag="rec", name="rec_t")
                    nc.vector.reciprocal_approx_fast(r["rec"][0:1, :r["w"]],
                                                     r["es"][0:1, :r["w"]])
                for r in rd:
                    r["rb"] = rbps.tile([P, REG], fp32, tag="rb", name="rb_t")
                    nc.tensor.matmul(r["rb"][:, :r["w"]], lhsT=ones1[:],
                                     rhs=r["rec"][0:1, :r["w"]], start=True, stop=True)
                for r in rd:
                    r["o"] = outp.tile([P, REG], fp32, tag="o", name="o_t")
                    nc.vector.tensor_tensor(out=r["o"][:, :r["w"]],
                                            in0=r["ex"][:, :r["w"]],
                                            in1=r["rb"][:, :r["w"]], op=Alu.mult)
                for q, r in enumerate(rd):
                    eng = nc.sync if q % 2 == 0 else nc.scalar
                    eng.dma_start(out_d[:, r["r0"]:r["r0"] + r["w"]],
                                  r["o"][:, :r["w"]])
    nc.compile()
    return nc


def kernel(x, ei, W):
    global _compiled
    in_maps, unshard = _host_prep(x, ei, W)
    if _compiled is None:
        _compiled = _build()
    nc = _compiled
    from concourse.bass_utils import run_bass_kernel_spmd
    res = run_bass_kernel_spmd(nc, in_maps, core_ids=list(range(NCORES)))
    out = np.empty(E, np.float32)
    for c in range(NCORES):
        order, starts_l, d, rowoff = unshard[c]
        o = res.results[c]["out"]
        for i in range(NLOC):
            n = order[i]
            dg = int(d[n])
            r = int(rowoff[i])
            out[starts_l[n]:starts_l[n] + dg] = o[r:r + dg, i]
    return out
